# revision 35
# baseline (speedup 1.0000x reference)
"""Trainium2 Bass kernel for nn_Actor_56916906607124 (compute_encoder_mask).

Computation (per batch instance b, row i):
  mask[b,i,j] = 1 iff  (j is among the 16 nearest time-window-compatible,
                        non-diagonal neighbors of i)  OR depot[b,i]  OR
                        depot[b,j]  OR i == j.

Sharding: pure data parallelism -- batch B=8 across 8 NeuronCores, one
instance per core.  No collectives.

Key structural facts exploited:
  * depot rows are all-ones and depot columns are all-ones in the output,
    independent of the KNN result.  Only non-depot rows (~1024 of 2048 per
    instance) need the device; the host memsets the rest while unsharding.
  * the selection key x = (twc && !diag) ? -d : -3 folds both inputs into a
    single bf16 tensor: eligible j have x = -d in (-1, 0], blocked j sit at
    -3, and the 16 nearest eligible neighbors are exactly the top-16 of x.
    bf16 rounding is monotone, so the bf16 top-16 equals the f32 top-16
    unless two values collide at the 16/17 boundary -- which the count
    check flags for exact host repair.

Per-core device program (R=1152 padded non-depot rows, 9 tiles of 128):
  DMA   : x tile [128,2048] bf16 in (4096 B/row descriptors, full rate).
  DVE   : folded = max(x[:, :1024], x[:, 1024:])  (bf16 2x mode; the Pool
          engine cannot run ALU ops on core v3);
          4x max8 over 256-wide chunks of folded -> 32 candidates;
          max8 -> top-8, match_replace, max8 -> ranks 9..16 => t16;
          bias = -t16 + eps;  is_ge count over the 896 non-stored cols
          (4x DVE mode: all-bf16 packed operands).
  ACT   : Sign(x + bias) SBUF->SBUF straight to uint8 over the 1152 stored
          cols (negatives wrap to 255; host maps ==1) with the accumulator
          shipping #sel - #unsel per row.
  DMA   : mask tile [128,1152] uint8 out on the scalar queue.

Host flags rows with count != 16 (boundary tie in bf16, fold collision, or
chunk-coverage miss -- any wrong t16 shifts the count off 16), t16 <= -2
(fewer than 16 eligible) or |t16| < 1e-3 (eps-guard margin), and recomputes
exactly those rows in f32 numpy.  ~950 of ~8100 rows on the seed-0 data;
verified to cover every differing row.
"""

from contextlib import ExitStack

import numpy as np

import concourse.bass as bass
import concourse.mybir as mybir
from concourse import bacc, tile

B, N, P = 8, 2048, 128
K = 16
EPS = 1e-7
f32 = mybir.dt.float32
bf16 = mybir.dt.bfloat16
u8 = mybir.dt.uint8
Alu = mybir.AluOpType
Act = mybir.ActivationFunctionType

_program_cache = {}


def build_program(rt=8, ct=9):
    """Device program for RT row-tiles of 128 non-depot rows; CT*128 stored
    (non-depot-first) columns."""
    key = ("nc", rt, ct)
    if key in _program_cache:
        return _program_cache[key]
    R = rt * P          # processed non-depot rows (leftover rows -> host)
    C = min(ct * P, N)  # stored (non-depot-first) columns
    REST = N - C        # trailing depot columns: counted, not stored
    K_ACT = max(0, rt - 2)  # tiles whose rest-count runs on ACT, not DVE

    nc = bacc.Bacc()
    x_h = nc.declare_dram_parameter("x", [R, N], bf16, isOutput=False)
    mask_h = nc.declare_dram_parameter("mask", [R, C], u8, isOutput=True)
    # last tile's mask is produced on DVE as bf16 0/1 (is_ge in 4x mode)
    # so the drain does not wait for the ACT engine
    maskl_h = nc.declare_dram_parameter("maskl", [P, C], bf16, isOutput=True)
    # stats columns (last tile's slots packed at the end so the bulk ships
    # before the drain): [0:rt-1] = rest-count tiles 0..rt-2,
    # [rt-1:2rt-2] = stored acc tiles 0..rt-2, [2rt-2:3rt-2] = ACT bias all
    # tiles (host recovers t16 ~ EPS - bias), [3rt-2] = rest-count last,
    # [3rt-1] = acc last, [3rt] = second-half acc of the split last tile
    stats_h = nc.declare_dram_parameter("stats", [P, 3 * rt + 1], f32,
                                        isOutput=True)

    def cnt_slot(r):
        return r if r < rt - 1 else 3 * rt - 2

    def acc_slot(r):
        return rt - 1 + r if r < rt - 1 else 3 * rt - 1

    def bias_slot(r):
        return 2 * rt - 2 + r

    H = N // 2
    with ExitStack() as ctx:
        tc = ctx.enter_context(tile.TileContext(nc))
        const = ctx.enter_context(tc.tile_pool(name="const", bufs=1))
        inp = ctx.enter_context(tc.tile_pool(name="inp", bufs=5))
        fold = ctx.enter_context(tc.tile_pool(name="fold", bufs=3))
        outp = ctx.enter_context(tc.tile_pool(name="outp", bufs=rt))
        small = ctx.enter_context(tc.tile_pool(name="small", bufs=4))
        junk = ctx.enter_context(tc.tile_pool(name="junk", bufs=2))

        v8ball = const.tile([P, 8 * rt], f32)
        stats_s = const.tile([P, 3 * rt + 1], f32)
        if REST and K_ACT:
            # ACT-offloaded tiles count the rest cols inside one full-width
            # Sign; their cnt slots are never written -- zero them so the
            # stats DMA does not ship uninitialized SBUF
            nc.gpsimd.memset(stats_s[:, 0 : min(K_ACT, rt - 1)], 0.0)
            if K_ACT == rt:
                nc.gpsimd.memset(
                    stats_s[:, 3 * rt - 2 : 3 * rt - 1], 0.0)

        pending_stores = []
        for r in range(rt):
            rows = slice(r * P, (r + 1) * P)
            x_t = inp.tile([P, N], bf16, tag="x")
            f_t = fold.tile([P, H], bf16, tag="f")
            if r == 0:
                # ramp: tile 0 loads in column pieces spread across both
                # HWDGE queues so the configs overlap, and fold1 runs in
                # halves so the DVE starts after the first two pieces land
                nc.sync.dma_start(x_t[:, 0:512], x_h[rows, 0:512])
                nc.scalar.dma_start(x_t[:, H : H + 512], x_h[rows, H : H + 512])
                nc.sync.dma_start(x_t[:, 512:H], x_h[rows, 512:H])
                nc.scalar.dma_start(x_t[:, H + 512 :], x_h[rows, H + 512 :])
                nc.vector.tensor_tensor(
                    f_t[:, 0:512], x_t[:, 0:512], x_t[:, H : H + 512], Alu.max)
                nc.vector.tensor_tensor(
                    f_t[:, 512:], x_t[:, 512:H], x_t[:, H + 512 :], Alu.max)
            else:
                nc.sync.dma_start(x_t[:], x_h[rows, :])
                # fold1[j] = max(x[j], x[j+1024]): any top-16 member of x
                # survives folding unless its partner also is one (fold
                # collision) -- then t16 comes out low and the count flags.
                nc.vector.tensor_tensor(
                    f_t[:], x_t[:, :H], x_t[:, H:], Alu.max)
            # fold2 (in place): slot j covers {j, j+512, j+1024, j+1536}
            nc.vector.tensor_tensor(
                f_t[:, 0:512], f_t[:, 0:512], f_t[:, 512:], Alu.max)
            # per-chunk top-8 of the 512 fold2 slots -> 32 candidates
            cand = small.tile([P, 32], f32, tag="cand")
            for c in range(4):
                nc.vector.max(cand[:, c * 8 : (c + 1) * 8],
                              f_t[:, c * 128 : (c + 1) * 128])
            v8a = small.tile([P, 8], f32, tag="v8a")
            nc.vector.max(v8a[:], cand[:])
            cand2 = small.tile([P, 32], f32, tag="cand2")
            nc.vector.match_replace(cand2[:], v8a[:], cand[:], -1e30)
            v8b = v8ball[:, r * 8 : (r + 1) * 8]
            nc.vector.max(v8b, cand2[:])
            t16 = v8ball[:, r * 8 + 7 : r * 8 + 8]
            # ACT bias: -t16 + EPS (EPS < any bf16 gap at |t16| >= 1e-3, so
            # Sign(x + bias) > 0  <=>  x >= t16; |t16| < 1e-3 rows flagged)
            bias = stats_s[:, bias_slot(r) : bias_slot(r) + 1]
            nc.vector.tensor_scalar(bias, t16, -1.0, EPS, Alu.mult, Alu.add)
            # stored mask: Sign gives 1 / 0 / -1(->255 as uint8); the
            # accumulator ships  #sel - #unsel  so count = (acc + width) / 2.
            # ACT-offloaded tiles Sign the FULL row in one pass (the [C:]
            # region is junk for the store but its accum IS the rest count);
            # the last tile runs in halves so its store drains while the
            # second half is still on the ACT engine.
            acc_ap = stats_s[:, acc_slot(r) : acc_slot(r) + 1]
            if r == rt - 1:
                # last tile: mask on DVE as bf16 0/1 (4x mode), accum is the
                # stored-col count directly; ACT plays no part in the drain
                outl_t = outp.tile([P, C], bf16, tag="outl")
                nc.vector.tensor_scalar(
                    outl_t[:], x_t[:, :C], t16, None, Alu.is_ge, Alu.add,
                    accum_out=acc_ap)
                nc.vector.memset(stats_s[:, 3 * rt : 3 * rt + 1], 0.0)
            elif REST and r < K_ACT:
                out_t = outp.tile([P, N], u8, tag="outw")
                nc.scalar.activation(out_t[:], x_t[:], Act.Sign, bias=bias,
                                     accum_out=acc_ap)
                pending_stores.append((rows, out_t))
            else:
                out_t = outp.tile([P, C], u8, tag="out")
                nc.scalar.activation(out_t[:], x_t[:, :C], Act.Sign, bias=bias,
                                     accum_out=acc_ap)
                pending_stores.append((rows, out_t))
            if not REST:
                nc.vector.memset(stats_s[:, cnt_slot(r) : cnt_slot(r) + 1], 0.0)
            elif r >= K_ACT:
                # late tiles keep the rest count on DVE (4x mode, in-place
                # over x) so the ACT stream ends earlier
                nc.vector.tensor_scalar(
                    x_t[:, C:], x_t[:, C:], t16, None, Alu.is_ge, Alu.add,
                    accum_out=stats_s[:, cnt_slot(r) : cnt_slot(r) + 1])

        # mask stores ride the sync queue AFTER every load: each store's
        # dependency wait blocks the issuing sequencer, so putting them on
        # the scalar queue would stall the next tile's Sign dispatch
        for rows, out_t in pending_stores:
            nc.sync.dma_start(mask_h[rows, :], out_t[:, :C])
        nc.sync.dma_start(maskl_h[:, :], outl_t[:])
        # stats ship in two pieces: the bulk (everything but the last tile's
        # slots, which sit contiguously at the end) leaves as soon as tile
        # rt-2 finishes; only 3 tail columns ride the drain path
        nc.scalar.dma_start(stats_h[:, : 3 * rt - 2],
                            stats_s[:, : 3 * rt - 2])
        nc.scalar.dma_start(stats_h[:, 3 * rt - 2 :],
                            stats_s[:, 3 * rt - 2 :])

    nc.compile()
    _program_cache[key] = nc
    return nc


def _repair_row(d_row, twc_row, depot_b, max_dist_b, i):
    """Exact float32 re-computation of reference row i (handles ties)."""
    n = d_row.shape[0]
    m = (twc_row == 0).astype(np.float32)
    m[i] = np.float32(1.0)
    big = (m * np.float32(max_dist_b)) * np.float32(10.0)
    dist = d_row * (np.float32(1.0) - m) + big
    idx = np.argsort(dist, kind="stable")[:K]
    knn = np.zeros(n, np.float32)
    knn[idx] = 1.0
    knn *= (twc_row == 1)
    dep = (depot_b + depot_b[i]) > 0
    out = ((knn > 0) | dep | (np.arange(n) == i)).astype(np.float32)
    return out


def _prep_core(d_b, twc_b, depot_b, rt, not_eye):
    """Build the per-core compacted selection-key tensor + index maps."""
    R = rt * P
    bf = mybir.dt.np(bf16)
    nd = np.flatnonzero(depot_b == 0)
    dep = np.flatnonzero(depot_b == 1)
    colperm = np.concatenate([nd, dep])
    xf = np.where((twc_b == 1) & not_eye, -d_b, np.float32(-3.0))
    xc = np.full((R, N), np.float32(-3.0), np.float32)
    nv = min(len(nd), R)
    xc[:nv] = xf[nd[:nv]][:, colperm]
    return xc.astype(bf), nd, colperm


def _get_executor(rt=8, ct=9):
    """Build the 8-core shard_map executable once (mirrors
    bass2jax.run_bass_via_pjrt, but cached so repeat calls skip retracing)."""
    key = ("exec", rt, ct)
    if key in _program_cache:
        return _program_cache[key]
    import jax
    from jax.sharding import Mesh, NamedSharding, PartitionSpec
    from jax.experimental.shard_map import shard_map
    from concourse import bass2jax
    from concourse.bass2jax import _bass_exec_p, install_neuronx_cc_hook

    nc = build_program(rt, ct)
    install_neuronx_cc_hook()
    partition_name = (nc.partition_id_tensor.name
                      if nc.partition_id_tensor else None)
    in_names, out_names, out_avals = [], [], []
    for alloc in nc.m.functions[0].allocations:
        if not isinstance(alloc, mybir.MemoryLocationSet):
            continue
        name = alloc.memorylocations[0].name
        if alloc.kind == "ExternalInput":
            if name != partition_name:
                in_names.append(name)
        elif alloc.kind == "ExternalOutput":
            out_names.append(name)
            out_avals.append(jax.core.ShapedArray(
                tuple(alloc.tensor_shape), mybir.dt.np(alloc.dtype)))
    all_in_names = list(in_names) + list(out_names)
    if partition_name is not None:
        all_in_names.append(partition_name)

    def _body(*args):
        operands = list(args)
        if partition_name is not None:
            operands.append(bass2jax.partition_id_tensor())
        return tuple(_bass_exec_p.bind(
            *operands,
            out_avals=tuple(out_avals),
            in_names=tuple(all_in_names),
            out_names=tuple(out_names),
            lowering_input_output_aliases=(),
            sim_require_finite=True,
            sim_require_nnan=True,
            nc=nc,
        ))

    devices = jax.devices()[:B]
    mesh = Mesh(np.asarray(devices), ("core",))
    spec = PartitionSpec("core")
    n_io = len(in_names) + len(out_names)
    sharded = jax.jit(
        shard_map(_body, mesh=mesh, in_specs=(spec,) * n_io,
                  out_specs=(spec,) * len(out_names), check_rep=False),
        donate_argnums=tuple(range(len(in_names), n_io)), keep_unused=True,
    )
    sharding = NamedSharding(mesh, spec)
    ex = (sharded, in_names, out_names, out_avals, sharding)
    _program_cache[key] = ex
    return ex


def _run_device(args_dev, rt, ct):
    import jax

    sharded, in_names, out_names, out_avals, sharding = _get_executor(rt, ct)
    # the kernel fully overwrites all outputs; donate last call's buffers
    prev = _program_cache.get(("outs", rt, ct))
    if prev is None:
        prev = tuple(jax.device_put(
            np.zeros((B * av.shape[0], *av.shape[1:]), av.dtype), sharding)
            for av in out_avals)
    outs_dev = sharded(*args_dev, *prev)
    _program_cache[("outs", rt, ct)] = outs_dev
    return {n: np.array(a).reshape(B, *out_avals[i].shape)
            for i, (n, a) in enumerate(zip(out_names, outs_dev))}


def kernel(distance_matrix, max_dist, time_window_compatibility, depot,
           num_neighbors_encoder):
    import jax

    distance_matrix = np.asarray(distance_matrix, dtype=np.float32)
    time_window_compatibility = np.asarray(time_window_compatibility,
                                           dtype=np.int32)
    depot = np.asarray(depot, dtype=np.int32)
    max_dist = np.asarray(max_dist, dtype=np.float32).reshape(B)
    assert int(np.asarray(num_neighbors_encoder)) == K
    assert distance_matrix.shape == (B, N, N)

    nd_counts = [(depot[b] == 0).sum() for b in range(B)]
    max_nd = int(max(nd_counts))
    ct = max(1, -(-max_nd // P))   # stored-column tiles (must cover nd cols)
    rt = ct
    if rt > 1 and max_nd - (rt - 1) * P <= 32:
        rt -= 1                    # leftover rows are cheaper on the host
    R_dev = rt * P                 # device-processed rows per core
    C = min(ct * P, N)
    REST = N - C
    K_ACT = max(0, rt - 2)         # must match build_program

    not_eye = ~np.eye(N, dtype=bool)
    preps = [_prep_core(distance_matrix[b], time_window_compatibility[b],
                        depot[b], rt, not_eye) for b in range(B)]
    sharded, in_names, out_names, out_avals, sharding = _get_executor(rt, ct)
    assert in_names == ["x"], in_names
    concat_x = np.concatenate([p[0] for p in preps], axis=0)
    args_dev = [jax.device_put(concat_x, sharding)]

    rng = np.random.default_rng(0)
    for attempt in range(3):
        by_name = _run_device(args_dev, rt, ct)
        raw = by_name["mask"]      # [B, R_dev, C] uint8: 1 sel, 0/255 unsel
        stats = by_name["stats"]   # [B, P, 3*rt+1]; layout per build_program
        cnt_rest = np.concatenate(
            [stats[:, :, : rt - 1], stats[:, :, 3 * rt - 2 : 3 * rt - 1]], -1)
        acc = np.concatenate(
            [stats[:, :, rt - 1 : 2 * rt - 2],
             stats[:, :, 3 * rt - 1 : 3 * rt]], -1).copy()
        acc[:, :, rt - 1] += stats[:, :, 3 * rt]  # split last tile
        t16 = np.float32(EPS) - stats[:, :, 2 * rt - 2 : 3 * rt - 2]
        # count over the whole row: ACT-offloaded tiles folded the rest cols
        # into one full-width accum (base N); DVE tiles ship the rest count
        # directly (base C)
        base = np.where((np.arange(rt) < K_ACT) & (REST > 0),
                        np.float32(N), np.float32(C))
        count_all = (acc + base) * np.float32(0.5) + cnt_rest
        # last tile's acc slot is a direct is_ge count, not #sel - #unsel
        count_all[:, :, rt - 1] = acc[:, :, rt - 1] + cnt_rest[:, :, rt - 1]

        out = np.zeros((B, N, N), np.float32)
        ar = np.arange(N)
        for b in range(B):
            _, nd, colperm = preps[b]
            RV = min(len(nd), R_dev)
            sel = (raw[b] == 1)
            sel[(rt - 1) * P :] = (by_name["maskl"][b] == 1.0)
            full = np.zeros((len(nd), N), np.float32)
            full[:RV, colperm[:C]] = sel[:RV]
            out[b, nd] = full
            dep_mask = depot[b] == 1
            out[b, dep_mask, :] = 1.0
            out[b, :, dep_mask] = 1.0
            out[b, ar, ar] = 1.0

            # exact repair of rows whose t16 is unreliable: count != 16
            # (bf16 tie at the 16/17 boundary, fold collision, or chunk
            # coverage miss all push the count off 16), < 16 eligible
            # neighbors (t16 = -3 sentinel), or |t16| below the eps guard.
            # Rows beyond the device's R_dev are computed here directly.
            count = count_all[b]
            rr = np.arange(RV)
            pp, tt = rr % P, rr // P
            bad = ((count[pp, tt] != np.float32(K))
                   | (t16[b][pp, tt] <= -1.5)
                   | (np.abs(t16[b][pp, tt]) < 1e-3))
            for r in list(np.flatnonzero(bad)) + list(range(RV, len(nd))):
                i = int(nd[r])
                out[b, i] = _repair_row(
                    distance_matrix[b, i], time_window_compatibility[b, i],
                    depot[b], max_dist[b], i,
                )

        # audit: recompute a random sample of rows exactly on host; any
        # mismatch indicates a transient device glitch -> rerun the call
        ok = True
        for _ in range(192):
            b = int(rng.integers(B))
            i = int(rng.integers(N))
            exp = _repair_row(
                distance_matrix[b, i], time_window_compatibility[b, i],
                depot[b], max_dist[b], i,
            )
            if not np.array_equal(out[b, i], exp):
                ok = False
                break
        if ok:
            return out
    return out


# revision 37
# speedup vs baseline: 1.0016x; 1.0016x over previous
"""Trainium2 Bass kernel for nn_Actor_56916906607124 (compute_encoder_mask).

Computation (per batch instance b, row i):
  mask[b,i,j] = 1 iff  (j is among the 16 nearest time-window-compatible,
                        non-diagonal neighbors of i)  OR depot[b,i]  OR
                        depot[b,j]  OR i == j.

Sharding: pure data parallelism -- batch B=8 across 8 NeuronCores, one
instance per core.  No collectives.

Key structural facts exploited:
  * depot rows are all-ones and depot columns are all-ones in the output,
    independent of the KNN result.  Only non-depot rows (~1024 of 2048 per
    instance) need the device; the host memsets the rest while unsharding.
  * the selection key x = (twc && !diag) ? -d : -3 folds both inputs into a
    single bf16 tensor: eligible j have x = -d in (-1, 0], blocked j sit at
    -3, and the 16 nearest eligible neighbors are exactly the top-16 of x.
    bf16 rounding is monotone, so the bf16 top-16 equals the f32 top-16
    unless two values collide at the 16/17 boundary -- which the count
    check flags for exact host repair.

Per-core device program (R=1152 padded non-depot rows, 9 tiles of 128):
  DMA   : x tile [128,2048] bf16 in (4096 B/row descriptors, full rate).
  DVE   : folded = max(x[:, :1024], x[:, 1024:])  (bf16 2x mode; the Pool
          engine cannot run ALU ops on core v3);
          4x max8 over 256-wide chunks of folded -> 32 candidates;
          max8 -> top-8, match_replace, max8 -> ranks 9..16 => t16;
          bias = -t16 + eps;  is_ge count over the 896 non-stored cols
          (4x DVE mode: all-bf16 packed operands).
  ACT   : Sign(x + bias) SBUF->SBUF straight to uint8 over the 1152 stored
          cols (negatives wrap to 255; host maps ==1) with the accumulator
          shipping #sel - #unsel per row.
  DMA   : mask tile [128,1152] uint8 out on the scalar queue.

Host flags rows with count != 16 (boundary tie in bf16, fold collision, or
chunk-coverage miss -- any wrong t16 shifts the count off 16), t16 <= -2
(fewer than 16 eligible) or |t16| < 1e-3 (eps-guard margin), and recomputes
exactly those rows in f32 numpy.  ~950 of ~8100 rows on the seed-0 data;
verified to cover every differing row.
"""

from contextlib import ExitStack

import numpy as np

import concourse.bass as bass
import concourse.mybir as mybir
from concourse import bacc, tile

B, N, P = 8, 2048, 128
K = 16
EPS = 1e-7
f32 = mybir.dt.float32
bf16 = mybir.dt.bfloat16
u8 = mybir.dt.uint8
Alu = mybir.AluOpType
Act = mybir.ActivationFunctionType

_program_cache = {}


def build_program(rt=8, ct=9):
    """Device program for RT row-tiles of 128 non-depot rows; CT*128 stored
    (non-depot-first) columns."""
    key = ("nc", rt, ct)
    if key in _program_cache:
        return _program_cache[key]
    R = rt * P          # processed non-depot rows (leftover rows -> host)
    C = min(ct * P, N)  # stored (non-depot-first) columns
    REST = N - C        # trailing depot columns: counted, not stored
    K_ACT = max(0, rt - 3)  # tiles whose rest-count runs on ACT, not DVE

    nc = bacc.Bacc()
    x_h = nc.declare_dram_parameter("x", [R, N], bf16, isOutput=False)
    mask_h = nc.declare_dram_parameter("mask", [R, C], u8, isOutput=True)
    # last tile's mask is produced on DVE as bf16 0/1 (is_ge in 4x mode)
    # so the drain does not wait for the ACT engine
    maskl_h = nc.declare_dram_parameter("maskl", [P, C], bf16, isOutput=True)
    # stats columns (last tile's slots packed at the end so the bulk ships
    # before the drain): [0:rt-1] = rest-count tiles 0..rt-2,
    # [rt-1:2rt-2] = stored acc tiles 0..rt-2, [2rt-2:3rt-2] = ACT bias all
    # tiles (host recovers t16 ~ EPS - bias), [3rt-2] = rest-count last,
    # [3rt-1] = acc last, [3rt] = second-half acc of the split last tile
    stats_h = nc.declare_dram_parameter("stats", [P, 3 * rt + 1], f32,
                                        isOutput=True)

    def cnt_slot(r):
        return r if r < rt - 1 else 3 * rt - 2

    def acc_slot(r):
        return rt - 1 + r if r < rt - 1 else 3 * rt - 1

    def bias_slot(r):
        return 2 * rt - 2 + r

    H = N // 2
    with ExitStack() as ctx:
        tc = ctx.enter_context(tile.TileContext(nc))
        const = ctx.enter_context(tc.tile_pool(name="const", bufs=1))
        inp = ctx.enter_context(tc.tile_pool(name="inp", bufs=5))
        fold = ctx.enter_context(tc.tile_pool(name="fold", bufs=3))
        outp = ctx.enter_context(tc.tile_pool(name="outp", bufs=rt))
        small = ctx.enter_context(tc.tile_pool(name="small", bufs=4))
        junk = ctx.enter_context(tc.tile_pool(name="junk", bufs=2))

        v8ball = const.tile([P, 8 * rt], f32)
        stats_s = const.tile([P, 3 * rt + 1], f32)
        if REST and K_ACT:
            # ACT-offloaded tiles count the rest cols inside one full-width
            # Sign; their cnt slots are never written -- zero them so the
            # stats DMA does not ship uninitialized SBUF
            nc.gpsimd.memset(stats_s[:, 0 : min(K_ACT, rt - 1)], 0.0)
            if K_ACT == rt:
                nc.gpsimd.memset(
                    stats_s[:, 3 * rt - 2 : 3 * rt - 1], 0.0)

        pending_stores = []
        for r in range(rt):
            rows = slice(r * P, (r + 1) * P)
            x_t = inp.tile([P, N], bf16, tag="x")
            f_t = fold.tile([P, H], bf16, tag="f")
            if r == 0:
                # ramp: tile 0 loads in column pieces spread across both
                # HWDGE queues so the configs overlap, and fold1 runs in
                # halves so the DVE starts after the first two pieces land
                nc.sync.dma_start(x_t[:, 0:512], x_h[rows, 0:512])
                nc.scalar.dma_start(x_t[:, H : H + 512], x_h[rows, H : H + 512])
                nc.sync.dma_start(x_t[:, 512:H], x_h[rows, 512:H])
                nc.scalar.dma_start(x_t[:, H + 512 :], x_h[rows, H + 512 :])
                nc.vector.tensor_tensor(
                    f_t[:, 0:512], x_t[:, 0:512], x_t[:, H : H + 512], Alu.max)
                nc.vector.tensor_tensor(
                    f_t[:, 512:], x_t[:, 512:H], x_t[:, H + 512 :], Alu.max)
            else:
                nc.sync.dma_start(x_t[:], x_h[rows, :])
                # fold1[j] = max(x[j], x[j+1024]): any top-16 member of x
                # survives folding unless its partner also is one (fold
                # collision) -- then t16 comes out low and the count flags.
                nc.vector.tensor_tensor(
                    f_t[:], x_t[:, :H], x_t[:, H:], Alu.max)
            # fold2 (in place): slot j covers {j, j+512, j+1024, j+1536}
            nc.vector.tensor_tensor(
                f_t[:, 0:512], f_t[:, 0:512], f_t[:, 512:], Alu.max)
            # per-chunk top-8 of the 512 fold2 slots -> 32 candidates
            cand = small.tile([P, 32], f32, tag="cand")
            for c in range(4):
                nc.vector.max(cand[:, c * 8 : (c + 1) * 8],
                              f_t[:, c * 128 : (c + 1) * 128])
            v8a = small.tile([P, 8], f32, tag="v8a")
            nc.vector.max(v8a[:], cand[:])
            cand2 = small.tile([P, 32], f32, tag="cand2")
            nc.vector.match_replace(cand2[:], v8a[:], cand[:], -1e30)
            v8b = v8ball[:, r * 8 : (r + 1) * 8]
            nc.vector.max(v8b, cand2[:])
            t16 = v8ball[:, r * 8 + 7 : r * 8 + 8]
            # ACT bias: -t16 + EPS (EPS < any bf16 gap at |t16| >= 1e-3, so
            # Sign(x + bias) > 0  <=>  x >= t16; |t16| < 1e-3 rows flagged)
            bias = stats_s[:, bias_slot(r) : bias_slot(r) + 1]
            nc.vector.tensor_scalar(bias, t16, -1.0, EPS, Alu.mult, Alu.add)
            # stored mask: Sign gives 1 / 0 / -1(->255 as uint8); the
            # accumulator ships  #sel - #unsel  so count = (acc + width) / 2.
            # ACT-offloaded tiles Sign the FULL row in one pass (the [C:]
            # region is junk for the store but its accum IS the rest count);
            # the last tile runs in halves so its store drains while the
            # second half is still on the ACT engine.
            acc_ap = stats_s[:, acc_slot(r) : acc_slot(r) + 1]
            if r == rt - 1:
                # last tile: mask on DVE as bf16 0/1 (4x mode), accum is the
                # stored-col count directly; ACT plays no part in the drain
                outl_t = outp.tile([P, C], bf16, tag="outl")
                nc.vector.tensor_scalar(
                    outl_t[:], x_t[:, :C], t16, None, Alu.is_ge, Alu.add,
                    accum_out=acc_ap)
                nc.vector.memset(stats_s[:, 3 * rt : 3 * rt + 1], 0.0)
            elif REST and r < K_ACT:
                out_t = outp.tile([P, N], u8, tag="outw")
                nc.scalar.activation(out_t[:], x_t[:], Act.Sign, bias=bias,
                                     accum_out=acc_ap)
                pending_stores.append((rows, out_t))
            else:
                out_t = outp.tile([P, C], u8, tag="out")
                nc.scalar.activation(out_t[:], x_t[:, :C], Act.Sign, bias=bias,
                                     accum_out=acc_ap)
                pending_stores.append((rows, out_t))
            if not REST:
                nc.vector.memset(stats_s[:, cnt_slot(r) : cnt_slot(r) + 1], 0.0)
            elif r >= K_ACT:
                # late tiles keep the rest count on DVE (4x mode, in-place
                # over x) so the ACT stream ends earlier
                nc.vector.tensor_scalar(
                    x_t[:, C:], x_t[:, C:], t16, None, Alu.is_ge, Alu.add,
                    accum_out=stats_s[:, cnt_slot(r) : cnt_slot(r) + 1])

        # mask stores ride the sync queue AFTER every load: each store's
        # dependency wait blocks the issuing sequencer, so putting them on
        # the scalar queue would stall the next tile's Sign dispatch
        for rows, out_t in pending_stores:
            nc.sync.dma_start(mask_h[rows, :], out_t[:, :C])
        nc.sync.dma_start(maskl_h[:, :], outl_t[:])
        # stats ship in two pieces: the bulk (everything but the last tile's
        # slots, which sit contiguously at the end) leaves as soon as tile
        # rt-2 finishes; only 3 tail columns ride the drain path
        nc.scalar.dma_start(stats_h[:, : 3 * rt - 2],
                            stats_s[:, : 3 * rt - 2])
        nc.scalar.dma_start(stats_h[:, 3 * rt - 2 :],
                            stats_s[:, 3 * rt - 2 :])

    nc.compile()
    _program_cache[key] = nc
    return nc


def _repair_row(d_row, twc_row, depot_b, max_dist_b, i):
    """Exact float32 re-computation of reference row i (handles ties)."""
    n = d_row.shape[0]
    m = (twc_row == 0).astype(np.float32)
    m[i] = np.float32(1.0)
    big = (m * np.float32(max_dist_b)) * np.float32(10.0)
    dist = d_row * (np.float32(1.0) - m) + big
    idx = np.argsort(dist, kind="stable")[:K]
    knn = np.zeros(n, np.float32)
    knn[idx] = 1.0
    knn *= (twc_row == 1)
    dep = (depot_b + depot_b[i]) > 0
    out = ((knn > 0) | dep | (np.arange(n) == i)).astype(np.float32)
    return out


def _prep_core(d_b, twc_b, depot_b, rt, not_eye):
    """Build the per-core compacted selection-key tensor + index maps."""
    R = rt * P
    bf = mybir.dt.np(bf16)
    nd = np.flatnonzero(depot_b == 0)
    dep = np.flatnonzero(depot_b == 1)
    colperm = np.concatenate([nd, dep])
    xf = np.where((twc_b == 1) & not_eye, -d_b, np.float32(-3.0))
    xc = np.full((R, N), np.float32(-3.0), np.float32)
    nv = min(len(nd), R)
    xc[:nv] = xf[nd[:nv]][:, colperm]
    return xc.astype(bf), nd, colperm


def _get_executor(rt=8, ct=9):
    """Build the 8-core shard_map executable once (mirrors
    bass2jax.run_bass_via_pjrt, but cached so repeat calls skip retracing)."""
    key = ("exec", rt, ct)
    if key in _program_cache:
        return _program_cache[key]
    import jax
    from jax.sharding import Mesh, NamedSharding, PartitionSpec
    from jax.experimental.shard_map import shard_map
    from concourse import bass2jax
    from concourse.bass2jax import _bass_exec_p, install_neuronx_cc_hook

    nc = build_program(rt, ct)
    install_neuronx_cc_hook()
    partition_name = (nc.partition_id_tensor.name
                      if nc.partition_id_tensor else None)
    in_names, out_names, out_avals = [], [], []
    for alloc in nc.m.functions[0].allocations:
        if not isinstance(alloc, mybir.MemoryLocationSet):
            continue
        name = alloc.memorylocations[0].name
        if alloc.kind == "ExternalInput":
            if name != partition_name:
                in_names.append(name)
        elif alloc.kind == "ExternalOutput":
            out_names.append(name)
            out_avals.append(jax.core.ShapedArray(
                tuple(alloc.tensor_shape), mybir.dt.np(alloc.dtype)))
    all_in_names = list(in_names) + list(out_names)
    if partition_name is not None:
        all_in_names.append(partition_name)

    def _body(*args):
        operands = list(args)
        if partition_name is not None:
            operands.append(bass2jax.partition_id_tensor())
        return tuple(_bass_exec_p.bind(
            *operands,
            out_avals=tuple(out_avals),
            in_names=tuple(all_in_names),
            out_names=tuple(out_names),
            lowering_input_output_aliases=(),
            sim_require_finite=True,
            sim_require_nnan=True,
            nc=nc,
        ))

    devices = jax.devices()[:B]
    mesh = Mesh(np.asarray(devices), ("core",))
    spec = PartitionSpec("core")
    n_io = len(in_names) + len(out_names)
    sharded = jax.jit(
        shard_map(_body, mesh=mesh, in_specs=(spec,) * n_io,
                  out_specs=(spec,) * len(out_names), check_rep=False),
        donate_argnums=tuple(range(len(in_names), n_io)), keep_unused=True,
    )
    sharding = NamedSharding(mesh, spec)
    ex = (sharded, in_names, out_names, out_avals, sharding)
    _program_cache[key] = ex
    return ex


def _run_device(args_dev, rt, ct):
    import jax

    sharded, in_names, out_names, out_avals, sharding = _get_executor(rt, ct)
    # the kernel fully overwrites all outputs; donate last call's buffers
    prev = _program_cache.get(("outs", rt, ct))
    if prev is None:
        prev = tuple(jax.device_put(
            np.zeros((B * av.shape[0], *av.shape[1:]), av.dtype), sharding)
            for av in out_avals)
    outs_dev = sharded(*args_dev, *prev)
    _program_cache[("outs", rt, ct)] = outs_dev
    return {n: np.array(a).reshape(B, *out_avals[i].shape)
            for i, (n, a) in enumerate(zip(out_names, outs_dev))}


def kernel(distance_matrix, max_dist, time_window_compatibility, depot,
           num_neighbors_encoder):
    import jax

    distance_matrix = np.asarray(distance_matrix, dtype=np.float32)
    time_window_compatibility = np.asarray(time_window_compatibility,
                                           dtype=np.int32)
    depot = np.asarray(depot, dtype=np.int32)
    max_dist = np.asarray(max_dist, dtype=np.float32).reshape(B)
    assert int(np.asarray(num_neighbors_encoder)) == K
    assert distance_matrix.shape == (B, N, N)

    nd_counts = [(depot[b] == 0).sum() for b in range(B)]
    max_nd = int(max(nd_counts))
    ct = max(1, -(-max_nd // P))   # stored-column tiles (must cover nd cols)
    rt = ct
    if rt > 1 and max_nd - (rt - 1) * P <= 32:
        rt -= 1                    # leftover rows are cheaper on the host
    R_dev = rt * P                 # device-processed rows per core
    C = min(ct * P, N)
    REST = N - C
    K_ACT = max(0, rt - 3)         # must match build_program

    not_eye = ~np.eye(N, dtype=bool)
    preps = [_prep_core(distance_matrix[b], time_window_compatibility[b],
                        depot[b], rt, not_eye) for b in range(B)]
    sharded, in_names, out_names, out_avals, sharding = _get_executor(rt, ct)
    assert in_names == ["x"], in_names
    concat_x = np.concatenate([p[0] for p in preps], axis=0)
    args_dev = [jax.device_put(concat_x, sharding)]

    rng = np.random.default_rng(0)
    for attempt in range(3):
        by_name = _run_device(args_dev, rt, ct)
        raw = by_name["mask"]      # [B, R_dev, C] uint8: 1 sel, 0/255 unsel
        stats = by_name["stats"]   # [B, P, 3*rt+1]; layout per build_program
        cnt_rest = np.concatenate(
            [stats[:, :, : rt - 1], stats[:, :, 3 * rt - 2 : 3 * rt - 1]], -1)
        acc = np.concatenate(
            [stats[:, :, rt - 1 : 2 * rt - 2],
             stats[:, :, 3 * rt - 1 : 3 * rt]], -1).copy()
        acc[:, :, rt - 1] += stats[:, :, 3 * rt]  # split last tile
        t16 = np.float32(EPS) - stats[:, :, 2 * rt - 2 : 3 * rt - 2]
        # count over the whole row: ACT-offloaded tiles folded the rest cols
        # into one full-width accum (base N); DVE tiles ship the rest count
        # directly (base C)
        base = np.where((np.arange(rt) < K_ACT) & (REST > 0),
                        np.float32(N), np.float32(C))
        count_all = (acc + base) * np.float32(0.5) + cnt_rest
        # last tile's acc slot is a direct is_ge count, not #sel - #unsel
        count_all[:, :, rt - 1] = acc[:, :, rt - 1] + cnt_rest[:, :, rt - 1]

        out = np.zeros((B, N, N), np.float32)
        ar = np.arange(N)
        for b in range(B):
            _, nd, colperm = preps[b]
            RV = min(len(nd), R_dev)
            sel = (raw[b] == 1)
            sel[(rt - 1) * P :] = (by_name["maskl"][b] == 1.0)
            full = np.zeros((len(nd), N), np.float32)
            full[:RV, colperm[:C]] = sel[:RV]
            out[b, nd] = full
            dep_mask = depot[b] == 1
            out[b, dep_mask, :] = 1.0
            out[b, :, dep_mask] = 1.0
            out[b, ar, ar] = 1.0

            # exact repair of rows whose t16 is unreliable: count != 16
            # (bf16 tie at the 16/17 boundary, fold collision, or chunk
            # coverage miss all push the count off 16), < 16 eligible
            # neighbors (t16 = -3 sentinel), or |t16| below the eps guard.
            # Rows beyond the device's R_dev are computed here directly.
            count = count_all[b]
            rr = np.arange(RV)
            pp, tt = rr % P, rr // P
            bad = ((count[pp, tt] != np.float32(K))
                   | (t16[b][pp, tt] <= -1.5)
                   | (np.abs(t16[b][pp, tt]) < 1e-3))
            for r in list(np.flatnonzero(bad)) + list(range(RV, len(nd))):
                i = int(nd[r])
                out[b, i] = _repair_row(
                    distance_matrix[b, i], time_window_compatibility[b, i],
                    depot[b], max_dist[b], i,
                )

        # audit: recompute a random sample of rows exactly on host; any
        # mismatch indicates a transient device glitch -> rerun the call
        ok = True
        for _ in range(192):
            b = int(rng.integers(B))
            i = int(rng.integers(N))
            exp = _repair_row(
                distance_matrix[b, i], time_window_compatibility[b, i],
                depot[b], max_dist[b], i,
            )
            if not np.array_equal(out[b, i], exp):
                ok = False
                break
        if ok:
            return out
    return out
